# revision 21
# baseline (speedup 1.0000x reference)
"""Trainium2 Bass kernel for nn_DotMatrix.

Math: for each (b, ell, t) the reference computes a complex pairwise dot
matrix O[i,j] = sum_m z[i,m] * w[j,m] where z = rep[b,:,t,:,:] as complex
and w the sign-flipped conjugation partner.  As a real matmul:

  lhsT[k, i]   k = (c,m) stacked: [Zr.T; Zi.T]                 [2m, 256]
  rhs[k, 2j+c'] c'=0: [FZr; -FZi], c'=1: [FZi; FZr]            [2m, 512]
  out = lhsT.T @ rhs  -> [256 i, 512 (j,c)]

with FZr[m',j] = s[m'] * Zr[j, M-1-m'], s[m'] = (-1)^(ell+m').

Precision: inputs and outputs travel as fp16 (e5m10).  fp16 x fp16
products accumulate exactly in f32 PSUM, so the end-to-end error is the
two roundings (~2^-11 in, ~2^-11 out): measured rel err 3.8e-4 against
the 2e-2 gate.  This leaves the kernel HBM-traffic minimal (no fp32
precision-stacking, K = 2m <= 14) and the wire cost at 1.57MB in +
4.72MB out per core.

Symmetry: the pairwise matrix is symmetric in (i,j) for both components
(O[i,j] = O[j,i]), so each channel computes 32-row i-blocks only against
j >= 32*bi (56.25% of the matrix) and the host mirrors the rest.

Sharding: 8 cores = 2 batches x 4 tau-quarters; each core owns 32
channels ch = (ell, slot) with t = tq*8 + slot.  The packed input
[128, 6144] holds 8 bands of 768 cols; band q carries 4 channels of one
ell, channel c4 in partition strip [32*c4 : 32*c4 + 2m] (lhsT 256 cols,
then rhs 512 cols).  A band is a "quad": its 4 channels run as 4
concurrent matmuls on diagonal PE tiles (32*c4, 32*c4) per i-block,
sharing one PSUM bank via column tiling.

Schedule (all per core, times approximate): engine init ends ~7us; the
first input chunk rides the sync HWDGE ring while the other three emit
concurrently from SWDGE (GpSimd), so band 0 lands ~10us.  A short chain
of dummy matmuls keeps the PE busy from engine start so the HAM clock
gate (1.2 -> 2.4 GHz after ~3.4us of continuous activity) releases just
as real matmuls begin.  Per quad, eight PSUM banks rotate through the
i-blocks; PSUM->SBUF evacuation (the per-quad bottleneck, ~1.6us) is
split by column count between VectorE (1216 cols) and ScalarE (1088
cols), casting f32 -> fp16 into a [128, 2304] staging tile.  Each quad
streams out as two stores - [0:1344] on the sync ring as soon as
i-blocks 0-2 land, the rest on the SWDGE queue - so the output stream
(~310 GB/s sustained) starts ~12.5us and overlaps all remaining
compute.  Host reassembles + mirrors into [2,256,256,128,2] f32.
"""

import numpy as np

import concourse.bacc as bacc
import concourse.mybir as mybir
from concourse.bass_utils import run_bass_kernel_spmd
from concourse.tile import TileContext

B, N, TAU, NELL = 2, 256, 32, 4
NCORES = 8
F32 = mybir.dt.float32
F16 = mybir.dt.float16
KS = [2 * (2 * ell + 1) for ell in range(NELL)]        # 2, 6, 10, 14
BIW = [512 - 64 * bi for bi in range(8)]               # cols per 32-row i-block
BIO = [0, 512, 960, 1344, 1664, 1920, 2112, 2240]      # ot offsets per i-block
OTW = 2304                                             # sum(BIW)
BANDS = [(0, 0), (3, 0), (2, 0), (1, 0), (3, 1), (2, 1), (1, 1), (0, 1)]
INW = 768 * len(BANDS)                                 # 6144

_NC_CACHE = {}


def _build_bass():
    nc = bacc.Bacc()
    inp_d = nc.declare_dram_parameter("inp", [128, INW], F16, isOutput=False)
    out = nc.declare_dram_parameter("out", [len(BANDS), 128, OTW], F16, isOutput=True)

    with TileContext(nc) as tc:
        with (
            tc.tile_pool(name="lin", bufs=1) as lin_pool,
            tc.tile_pool(name="ps", bufs=1, space="PSUM") as ps_pool,
            tc.tile_pool(name="ot", bufs=6) as ot_pool,
        ):
            in_sb = lin_pool.tile([128, INW], F16, name="in_sb")
            # PE pre-warm: the HAM clock gate releases (1.2 -> 2.4 GHz) only
            # after ~3.4us of continuous PE activity.  A few dependency-free
            # dummy matmuls bridge the gap from engine start to the first
            # real matmul (band 0 is tiny and lands early); the real quads
            # then keep the PE busy through the HAM flip.
            warm_in = lin_pool.tile([128, 256], F16, name="warm_in")
            warm_ps = ps_pool.tile([128, 512], F32, tag="psb", bufs=8, name="warm_ps")
            nc.gpsimd.memset(warm_in[:], 0.0)
            for _ in range(14):
                nc.tensor.matmul(
                    warm_ps[:, 0:256], warm_in[:, 0:128], warm_in[:, 0:256],
                    start=True, stop=True,
                )
            # Input rides SWDGE (GpSimd): that engine comes out of kernel
            # init ~1.4us before the HWDGE engines, and it keeps both HWDGE
            # rings free so the sync ring carries nothing but output stores.
            # Four chunks so early quads unblock while later bands stream.
            nc.sync.dma_start(out=in_sb[:, 0:768], in_=inp_d[:, 0:768])
            nc.gpsimd.dma_start(out=in_sb[:, 768:2304], in_=inp_d[:, 768:2304])
            nc.gpsimd.dma_start(out=in_sb[:, 2304:3840], in_=inp_d[:, 2304:3840])
            nc.gpsimd.dma_start(out=in_sb[:, 3840:6144], in_=inp_d[:, 3840:6144])
            # Evacuation pairs adjacent i-blocks into shared PSUM tiles so
            # the PSUM->SBUF copies are 5 big ops/quad instead of 8 (per-op
            # overhead is ~15-25% of evac time).  Pairs land contiguously in
            # PSUM ((bi0@0, bi1@512) spans two banks; (bi4@0, bi5@256) and
            # (bi6@0, bi7@128) share a bank) so one copy drains both.
            # DVE takes 1216 cols/quad, ACT 1088 (ACT is a bit slower).
            evac_dve = [True, False, True, False, True, False, False, True]
            def mm_block(ps, q, e, bi, col0):
                K, W, base = KS[e], BIW[bi], q * 768
                for c4 in range(4):  # channel within quad
                    r0 = 32 * c4
                    nc.tensor.matmul(
                        ps[r0 : r0 + 32, col0 : col0 + W],
                        in_sb[r0 : r0 + K, base + bi * 32 : base + bi * 32 + 32],
                        in_sb[r0 : r0 + K, base + 256 + 64 * bi : base + 768],
                        start=True,
                        stop=True,
                        tile_position=(r0, r0),
                    )

            for q, (e, v) in enumerate(BANDS):
                ot = ot_pool.tile([128, OTW], F16)
                for bi in range(8):     # i-block of 32 rows
                    W = BIW[bi]
                    ps = ps_pool.tile([128, 512], F32, tag="psb", bufs=8)
                    mm_block(ps, q, e, bi, 0)
                    dst = ot[:, BIO[bi] : BIO[bi] + W]
                    if evac_dve[bi]:
                        nc.vector.tensor_copy(out=dst, in_=ps[:, 0:W])
                    else:
                        nc.scalar.copy(dst, ps[:, 0:W])
                    if bi == 3:
                        # first half of the quad (i-blocks 0-3) streams out
                        # while the second half is still computing
                        nc.sync.dma_start(out=out[q, :, 0:1344], in_=ot[:, 0:1344])
                # second half rides the SWDGE queue: two descriptor streams
                # keep the SDMA engines fed across store boundaries
                if q < 7:
                    nc.gpsimd.dma_start(out=out[q, :, 1344:OTW], in_=ot[:, 1344:OTW])
                else:
                    # split the very last store so the end-of-kernel barrier
                    # only waits on a 49KB receipt
                    nc.gpsimd.dma_start(out=out[q, :, 1344:2112], in_=ot[:, 1344:2112])
                    nc.sync.dma_start(out=out[q, :, 2112:OTW], in_=ot[:, 2112:OTW])
    nc.compile()
    return nc


def _host_prep(reps, cid):
    """Build the per-core packed fp16 input [128, 6144]."""
    b, tq = cid // 4, cid % 4
    INP = np.zeros((128, INW), np.float16)
    for q, (e, v) in enumerate(BANDS):
        m = 2 * e + 1
        K = KS[e]
        s_vec = ((-1.0) ** (e + np.arange(m))).astype(np.float32)
        for c4 in range(4):
            t = tq * 8 + 4 * v + c4
            Z = reps[e][b, :, t]                  # [256, m, 2]
            Zr, Zi = Z[..., 0], Z[..., 1]         # [256, m]
            lhsT = np.concatenate([Zr.T, Zi.T], axis=0)      # [2m, 256]
            FZr = s_vec[:, None] * Zr[:, ::-1].T             # [m, 256]
            FZi = s_vec[:, None] * Zi[:, ::-1].T
            R = np.empty((K, 256, 2), np.float32)
            R[0:m, :, 0] = FZr
            R[m:, :, 0] = -FZi
            R[0:m, :, 1] = FZi
            R[m:, :, 1] = FZr
            r0 = 32 * c4
            INP[r0 : r0 + K, q * 768 : q * 768 + 256] = lhsT
            INP[r0 : r0 + K, q * 768 + 256 : q * 768 + 768] = R.reshape(K, 512)
    return {"inp": INP}


def _run(in_maps, **kw):
    if "nc" not in _NC_CACHE:
        _NC_CACHE["nc"] = _build_bass()
    return run_bass_kernel_spmd(_NC_CACHE["nc"], in_maps, list(range(NCORES)), **kw)


def kernel(rep0, rep1, rep2, rep3, _bass_kw=None):
    reps = [np.ascontiguousarray(np.asarray(r, dtype=np.float32)) for r in (rep0, rep1, rep2, rep3)]
    in_maps = [_host_prep(reps, cid) for cid in range(NCORES)]
    res = _run(in_maps, **(_bass_kw or {}))
    out = np.empty((B, N, N, NELL * TAU, 2), np.float32)
    for cid in range(NCORES):
        b, tq = cid // 4, cid % 4
        arr = np.asarray(res.results[cid]["out"]).astype(np.float32)  # [8, 128, OTW]
        for q, (e, v) in enumerate(BANDS):
            och = np.empty((4, 256, 256, 2), np.float32)   # [c4, i, j, comp]
            for bi in range(8):
                nj = 256 - 32 * bi
                blk = arr[q, :, BIO[bi] : BIO[bi] + BIW[bi]].reshape(4, 32, nj, 2)
                och[:, 32 * bi : 32 * bi + 32, 32 * bi :, :] = blk
            for bi in range(1, 8):                  # mirror lower block triangle
                r = slice(32 * bi, 32 * bi + 32)
                och[:, r, : 32 * bi, :] = och[:, : 32 * bi, r, :].transpose(0, 2, 1, 3)
            lo = e * TAU + tq * 8 + 4 * v
            out[b, :, :, lo : lo + 4, :] = och.transpose(1, 2, 0, 3)
    kernel.last_result = res
    return out
